# revision 11
# baseline (speedup 1.0000x reference)
"""Bidirectional LSTM kernel for Trainium2 (Bass/Tile), B=64 S=256 I=H=512.

Strategy:
- Core 0 runs the forward direction, core 1 the backward direction (same SPMD
  program; the host reverses time for core 1's inputs/outputs).
- Transposed ("gates^T") layout: the recurrent GEMM keeps the 64 Wh weight
  tiles stationary on the PE array and streams h^T (512x64) as the moving
  operand, producing gates^T (2048x64) in PSUM.  The elementwise cell update
  then runs on full 128-partition tiles and produces h^T directly in the
  layout the next step's GEMM consumes - no per-step transpose.
- The input projection x@Wx + b is computed in 16-step sweep windows into an
  SBUF ring buffer (amortized weight loads), and injected into each step's
  PSUM accumulation via identity-matmul preloads, so the per-step critical
  path never touches it.
"""

import numpy as np
import ml_dtypes

P = 128
B = 64          # batch
HD = 512        # hidden dim
ID = 512        # input dim
KH = HD // P    # 4 k-chunks over h
KI = ID // P    # 4 k-chunks over x
M4 = 4 * HD // P  # 16 m-chunks over the 4*H gate dim; order [g, i, f, o]
S_FULL = 256
SWEEP_FULL = 16

_NC_CACHE = {}


def build(S=S_FULL, SWEEP=SWEEP_FULL):
    """Build and bacc-compile the single-core LSTM program."""
    import concourse.bacc as bacc
    import concourse.mybir as mybir
    import concourse.tile as tile
    from concourse.tile import add_dep_helper
    from contextlib import ExitStack

    AF = mybir.ActivationFunctionType
    bf16 = mybir.dt.bfloat16
    f32 = mybir.dt.float32

    assert S % SWEEP == 0
    n_sweeps = S // SWEEP
    COLS = SWEEP * B              # columns per sweep window
    NCH = max(1, COLS // 512)     # 512-col chunks per window
    NCOL = COLS // NCH            # columns per chunk (<= 512)
    TPC = NCOL // B               # timesteps covered per chunk
    n_groups = NCH * M4           # (n, m) GEMM groups per window
    assert n_groups % SWEEP == 0
    gps = n_groups // SWEEP       # groups emitted per step

    nc = bacc.Bacc("TRN2", target_bir_lowering=False, debug=False, num_devices=2)

    xT = nc.dram_tensor("xT", (P, KI, S * B), bf16, kind="ExternalInput")
    wx = nc.dram_tensor("wx", (P, KI, M4, P), bf16, kind="ExternalInput")
    wh = nc.dram_tensor("wh", (P, KH, M4, P), bf16, kind="ExternalInput")
    bias = nc.dram_tensor("bias", (P, M4), f32, kind="ExternalInput")
    ident = nc.dram_tensor("ident", (P, P), bf16, kind="ExternalInput")
    hsT = nc.dram_tensor("hsT", (S, KH, P, B), bf16, kind="ExternalOutput")

    with tile.TileContext(nc) as tc, ExitStack() as ctx:
        constp = ctx.enter_context(tc.tile_pool(name="const", bufs=1))
        xinp = ctx.enter_context(tc.tile_pool(name="xin", bufs=2))
        ringp = ctx.enter_context(tc.tile_pool(name="ring", bufs=2))
        statep = ctx.enter_context(tc.tile_pool(name="state", bufs=3))
        ewp = ctx.enter_context(tc.tile_pool(name="ew", bufs=3))
        psga = ctx.enter_context(tc.tile_pool(name="psum_ga", bufs=2, space="PSUM"))
        psgb = ctx.enter_context(tc.tile_pool(name="psum_gb", bufs=2, space="PSUM"))
        psgc = ctx.enter_context(tc.tile_pool(name="psum_gc", bufs=2, space="PSUM"))
        psx = ctx.enter_context(tc.tile_pool(name="psum_x", bufs=2, space="PSUM"))

        wx_sb = constp.tile([P, KI, M4, P], bf16)
        wh_sb = constp.tile([P, KH, M4, P], bf16)
        for k in range(KI):
            nc.sync.dma_start(out=wx_sb[:, k], in_=wx.ap()[:, k])
        for k in range(KH):
            nc.sync.dma_start(out=wh_sb[:, k], in_=wh.ap()[:, k])
        bias_sb = constp.tile([P, M4], f32)
        nc.sync.dma_start(out=bias_sb[:], in_=bias.ap())
        id_sb = constp.tile([P, P], bf16)
        nc.sync.dma_start(out=id_sb[:], in_=ident.ap())

        x_bufs = {}
        ring_bufs = {}

        def load_x(s):
            t_ = xinp.tile([P, KI, COLS], bf16, tag="xin", name=f"xin{s}")
            nc.sync.dma_start(out=t_[:], in_=xT.ap()[:, :, s * COLS:(s + 1) * COLS])
            x_bufs[s] = t_

        def new_ring(s):
            ring_bufs[s] = ringp.tile([P, SWEEP, M4, B], bf16, tag="ring", name=f"ring{s}")

        def sweep_group(s, n, m, after=None):
            # x-projection GEMM for sweep window s, column-chunk n, m-chunk m.
            # `after`: PE instruction to order the first matmul behind
            # (ordering-only dep, same engine) so sweeps land in step tails.
            xb = x_bufs[s]
            rb = ring_bufs[s]
            pt = psx.tile([P, TPC, B], f32, tag="psx")
            last = None
            for k in range(KI):
                mm = nc.tensor.matmul(
                    pt[:], wx_sb[:, k, m, :], xb[:, k, n * NCOL:(n + 1) * NCOL],
                    start=(k == 0), stop=(k == KI - 1),
                )
                if k == 0 and after is not None:
                    add_dep_helper(mm.ins, after.ins, sync=False,
                                   reason="pin sweep into step tail")
                last = mm
            # evict to ring with the gate bias folded in (per-partition bias);
            # on DVE to keep the ScalarE free for the cell-update chain
            nc.vector.tensor_scalar_add(
                out=rb[:, n * TPC:(n + 1) * TPC, m, :], in0=pt[:],
                scalar1=bias_sb[:, m:m + 1],
            )
            return last

        # prologue: first sweep window fully
        load_x(0)
        new_ring(0)
        for n in range(NCH):
            for m in range(M4):
                sweep_group(0, n, m)

        h_prev = None
        c_prev = None
        last_sweep_mm = None   # last sweep matmul of the previous step
        MH = M4 // 2
        for t in range(S):
            s, sl = divmod(t, SWEEP)
            if s + 1 < n_sweeps and sl == 0:
                load_x(s + 1)
                new_ring(s + 1)

            rb = ring_bufs[s]
            # Three PSUM tiles (one bank each): A holds gates g,i (m 0-7),
            # B holds f (m 8-11), C holds o (m 12-15).  Separate banks mean
            # each tile's activations wait only on that tile's matmuls.
            gpa = psga.tile([P, MH, B], f32, tag="ga")
            gpb = psgb.tile([P, KH, B], f32, tag="gb")
            gpc = psgc.tile([P, KH, B], f32, tag="gc")

            def gp_slot(m):
                if m < MH:
                    return gpa, m, MH
                if m < MH + KH:
                    return gpb, m - MH, KH
                return gpc, m - MH - KH, KH

            # PSUM accumulation groups are 2KB-bank granular: start=True marks
            # the bank lazily-zero (first writer of each byte overwrites, later
            # writers accumulate); stop goes on the bank's last matmul.
            first_pre = None
            for m in range(M4):
                gp_t, ml, nl = gp_slot(m)
                mm = nc.tensor.matmul(gp_t[:, ml, :], id_sb[:], rb[:, sl, m, :],
                                      start=(ml == 0),
                                      stop=(t == 0 and ml == nl - 1))
                if m == 0:
                    first_pre = mm
            if last_sweep_mm is not None:
                # keep the PE stream interleaved: this step's preloads run
                # after the previous step's sweep work (ordering-only)
                add_dep_helper(first_pre.ins, last_sweep_mm.ins, sync=False,
                               reason="preloads after prior step sweeps")
            last_h_mm = first_pre
            if t > 0:
                for m in range(M4):
                    gp_t, ml, nl = gp_slot(m)
                    for k in range(KH):
                        last_h_mm = nc.tensor.matmul(
                            gp_t[:, ml, :], wh_sb[:, k, m, :], h_prev[:, k, :],
                            start=False,
                            stop=(k == KH - 1 and ml == nl - 1))

            # elementwise cell update; gate m-chunk order is [g, i | f | o]
            tg = ewp.tile([P, KH, B], bf16, tag="tg")
            nc.scalar.activation(tg[:], gpa[:, 0:KH, :], AF.Tanh)
            si = ewp.tile([P, KH, B], bf16, tag="si")
            nc.scalar.activation(si[:], gpa[:, KH:2 * KH, :], AF.Sigmoid)
            t1 = ewp.tile([P, KH, B], f32, tag="t1")
            nc.vector.tensor_mul(out=t1[:], in0=si[:], in1=tg[:])

            sf = ewp.tile([P, KH, B], bf16, tag="sf")
            nc.scalar.activation(sf[:], gpb[:], AF.Sigmoid)
            so = ewp.tile([P, KH, B], bf16, tag="so")
            nc.scalar.activation(so[:], gpc[:], AF.Sigmoid)

            c_new = statep.tile([P, KH, B], f32, tag="c")
            if t == 0:
                nc.vector.tensor_copy(out=c_new[:], in_=t1[:])
            else:
                t2 = ewp.tile([P, KH, B], f32, tag="t2")
                nc.vector.tensor_mul(out=t2[:], in0=sf[:], in1=c_prev[:])
                nc.vector.tensor_add(out=c_new[:], in0=t1[:], in1=t2[:])
            tct = ewp.tile([P, KH, B], bf16, tag="tct")
            nc.scalar.activation(tct[:], c_new[:], AF.Tanh)
            h_new = statep.tile([P, KH, B], bf16, tag="hT")
            nc.vector.tensor_mul(out=h_new[:], in0=so[:], in1=tct[:])
            nc.sync.dma_start(out=hsT.ap()[t].rearrange("k p b -> p k b"), in_=h_new[:])

            h_prev, c_prev = h_new, c_new

            # emit next window's x-projection groups, pinned behind this step's
            # last recurrent matmul so they fill this step's PE idle tail
            if s + 1 < n_sweeps:
                for g in range(sl * gps, (sl + 1) * gps):
                    last_sweep_mm = sweep_group(s + 1, g // M4, g % M4,
                                                after=last_h_mm)
            else:
                last_sweep_mm = None

    nc.compile()
    return nc


def _get_nc(S, SWEEP):
    key = (S, SWEEP)
    if key not in _NC_CACHE:
        _NC_CACHE[key] = build(S, SWEEP)
    return _NC_CACHE[key]


def prep_core_inputs(x, Wc, bc, Wi, bi, Wf, bf, Wo, bo, reverse):
    """Pack one direction's inputs into the kernel's layouts. x: (B, S, I) f32."""
    bft = ml_dtypes.bfloat16
    if reverse:
        x = x[:, ::-1, :]
    S = x.shape[1]
    Wcat = np.concatenate([Wc, Wi, Wf, Wo], axis=1)      # (I+H, 4H), gate order [g,i,f,o]
    bcat = np.concatenate([bc, bi, bf, bo]).astype(np.float32)
    Wx, Wh = Wcat[:ID], Wcat[ID:]

    xT = (
        x.transpose(2, 1, 0)                  # (I, S, B)
        .reshape(KI, P, S * B)
        .transpose(1, 0, 2)                   # (P, KI, S*B)
    )
    wxp = Wx.reshape(KI, P, M4, P).transpose(1, 0, 2, 3)
    whp = Wh.reshape(KH, P, M4, P).transpose(1, 0, 2, 3)
    biasp = bcat.reshape(M4, P).T
    return {
        "xT": np.ascontiguousarray(xT).astype(bft),
        "wx": np.ascontiguousarray(wxp).astype(bft),
        "wh": np.ascontiguousarray(whp).astype(bft),
        "bias": np.ascontiguousarray(biasp),
        "ident": np.eye(P, dtype=bft),
    }


def run_lstm(x, Wi_f, bi_f, Wf_f, bf_f, Wc_f, bc_f, Wo_f, bo_f,
             Wi_b, bi_b, Wf_b, bf_b, Wc_b, bc_b, Wo_b, bo_b,
             trace=False, trace_cores=None):
    from concourse import bass_utils

    x = np.asarray(x, dtype=np.float32)
    S = x.shape[1]
    nc = _get_nc(S, SWEEP_FULL if S % SWEEP_FULL == 0 else S)
    im0 = prep_core_inputs(x, Wc_f, bc_f, Wi_f, bi_f, Wf_f, bf_f, Wo_f, bo_f, False)
    im1 = prep_core_inputs(x, Wc_b, bc_b, Wi_b, bi_b, Wf_b, bf_b, Wo_b, bo_b, True)
    res = bass_utils.run_bass_kernel_spmd(
        nc, [im0, im1], core_ids=[0, 1], trace=trace, trace_cores=trace_cores,
    )
    hsf = res.results[0]["hsT"].astype(np.float32)    # (S, KH, P, B)
    hsb = res.results[1]["hsT"][::-1].astype(np.float32)
    fwd = hsf.transpose(0, 3, 1, 2).reshape(S, B, HD)   # (S, B, H)
    bwd = hsb.transpose(0, 3, 1, 2).reshape(S, B, HD)
    out = np.concatenate([fwd, bwd], axis=2).transpose(1, 0, 2)  # (B, S, 2H)
    return np.ascontiguousarray(out), res


def kernel(x, Wi_f, bi_f, Wf_f, bf_f, Wc_f, bc_f, Wo_f, bo_f,
           Wi_b, bi_b, Wf_b, bf_b, Wc_b, bc_b, Wo_b, bo_b):
    out, _ = run_lstm(x, Wi_f, bi_f, Wf_f, bf_f, Wc_f, bc_f, Wo_f, bo_f,
                      Wi_b, bi_b, Wf_b, bf_b, Wc_b, bc_b, Wo_b, bo_b)
    return out


# revision 12
# speedup vs baseline: 1.1020x; 1.1020x over previous
"""Bidirectional LSTM kernel for Trainium2 (Bass/Tile), B=64 S=256 I=H=512.

Strategy:
- Core 0 runs the forward direction, core 1 the backward direction (same SPMD
  program; the host reverses time for core 1's inputs/outputs).
- Transposed ("gates^T") layout: the recurrent GEMM keeps the 64 Wh weight
  tiles stationary on the PE array and streams h^T (512x64) as the moving
  operand, producing gates^T (2048x64) in PSUM.  The elementwise cell update
  then runs on full 128-partition tiles and produces h^T directly in the
  layout the next step's GEMM consumes - no per-step transpose.
- The input projection x@Wx + b is computed in 16-step sweep windows into an
  SBUF ring buffer (amortized weight loads), and injected into each step's
  PSUM accumulation via identity-matmul preloads, so the per-step critical
  path never touches it.
"""

import numpy as np
import ml_dtypes

P = 128
B = 64          # batch
HD = 512        # hidden dim
ID = 512        # input dim
KH = HD // P    # 4 k-chunks over h
KI = ID // P    # 4 k-chunks over x
M4 = 4 * HD // P  # 16 m-chunks over the 4*H gate dim; order [g, i, f, o]
S_FULL = 256
SWEEP_FULL = 16

_NC_CACHE = {}


def build(S=S_FULL, SWEEP=SWEEP_FULL):
    """Build and bacc-compile the single-core LSTM program."""
    import concourse.bacc as bacc
    import concourse.mybir as mybir
    import concourse.tile as tile
    from concourse.tile import add_dep_helper
    from contextlib import ExitStack

    AF = mybir.ActivationFunctionType
    bf16 = mybir.dt.bfloat16
    f32 = mybir.dt.float32

    assert S % SWEEP == 0
    n_sweeps = S // SWEEP
    COLS = SWEEP * B              # columns per sweep window
    NCH = max(1, COLS // 512)     # 512-col chunks per window
    NCOL = COLS // NCH            # columns per chunk (<= 512)
    TPC = NCOL // B               # timesteps covered per chunk
    n_groups = NCH * M4           # (n, m) GEMM groups per window
    assert n_groups % SWEEP == 0
    gps = n_groups // SWEEP       # groups emitted per step

    nc = bacc.Bacc("TRN2", target_bir_lowering=False, debug=False, num_devices=2)

    xT = nc.dram_tensor("xT", (P, KI, S * B), bf16, kind="ExternalInput")
    wx = nc.dram_tensor("wx", (P, KI, M4, P), bf16, kind="ExternalInput")
    wh = nc.dram_tensor("wh", (P, KH, M4, P), bf16, kind="ExternalInput")
    bias = nc.dram_tensor("bias", (P, M4), f32, kind="ExternalInput")
    ident = nc.dram_tensor("ident", (P, P), bf16, kind="ExternalInput")
    hsT = nc.dram_tensor("hsT", (S, KH, P, B), bf16, kind="ExternalOutput")

    with tile.TileContext(nc) as tc, ExitStack() as ctx:
        constp = ctx.enter_context(tc.tile_pool(name="const", bufs=1))
        xinp = ctx.enter_context(tc.tile_pool(name="xin", bufs=2))
        ringp = ctx.enter_context(tc.tile_pool(name="ring", bufs=2))
        statep = ctx.enter_context(tc.tile_pool(name="state", bufs=3))
        ewp = ctx.enter_context(tc.tile_pool(name="ew", bufs=3))
        psga = ctx.enter_context(tc.tile_pool(name="psum_ga", bufs=2, space="PSUM"))
        psgb = ctx.enter_context(tc.tile_pool(name="psum_gb", bufs=1, space="PSUM"))
        psgc = ctx.enter_context(tc.tile_pool(name="psum_gc", bufs=1, space="PSUM"))
        psx = ctx.enter_context(tc.tile_pool(name="psum_x", bufs=4, space="PSUM"))

        wx_sb = constp.tile([P, KI, M4, P], bf16)
        wh_sb = constp.tile([P, KH, M4, P], bf16)
        for k in range(KI):
            nc.sync.dma_start(out=wx_sb[:, k], in_=wx.ap()[:, k])
        for k in range(KH):
            nc.sync.dma_start(out=wh_sb[:, k], in_=wh.ap()[:, k])
        bias_sb = constp.tile([P, M4], f32)
        nc.sync.dma_start(out=bias_sb[:], in_=bias.ap())
        id_sb = constp.tile([P, P], bf16)
        nc.sync.dma_start(out=id_sb[:], in_=ident.ap())

        x_bufs = {}
        ring_bufs = {}

        def load_x(s):
            t_ = xinp.tile([P, KI, COLS], bf16, tag="xin", name=f"xin{s}")
            nc.sync.dma_start(out=t_[:], in_=xT.ap()[:, :, s * COLS:(s + 1) * COLS])
            x_bufs[s] = t_

        def new_ring(s):
            ring_bufs[s] = ringp.tile([P, SWEEP, M4, B], bf16, tag="ring", name=f"ring{s}")

        def sweep_group(s, n, m, after=None, evict_dve=False, evict_after=None):
            # x-projection GEMM for sweep window s, column-chunk n, m-chunk m.
            # `after`: PE instruction to order the first matmul behind
            # (ordering-only dep, same engine) so sweeps land in step tails.
            xb = x_bufs[s]
            rb = ring_bufs[s]
            pt = psx.tile([P, TPC, B], f32, tag="psx")
            last = None
            for k in range(KI):
                mm = nc.tensor.matmul(
                    pt[:], wx_sb[:, k, m, :], xb[:, k, n * NCOL:(n + 1) * NCOL],
                    start=(k == 0), stop=(k == KI - 1),
                )
                if k == 0 and after is not None:
                    add_dep_helper(mm.ins, after.ins, sync=False,
                                   reason="pin sweep into step tail")
                last = mm
            # evict to ring with the gate bias folded in (per-partition bias);
            # alternate between DVE and ScalarE to balance engine load
            if evict_dve:
                ev = nc.vector.tensor_scalar_add(
                    out=rb[:, n * TPC:(n + 1) * TPC, m, :], in0=pt[:],
                    scalar1=bias_sb[:, m:m + 1],
                )
            else:
                ev = nc.scalar.activation(
                    rb[:, n * TPC:(n + 1) * TPC, m, :], pt[:],
                    AF.Identity, bias=bias_sb[:, m:m + 1],
                )
            if evict_after is not None:
                add_dep_helper(ev.ins, evict_after.ins, sync=False,
                               reason="evict after step chain ops")
            return last

        # prologue: first sweep window fully
        load_x(0)
        new_ring(0)
        for n in range(NCH):
            for m in range(M4):
                sweep_group(0, n, m)

        h_prev = None
        c_prev = None
        last_sweep_mm = None   # last sweep matmul of the previous step
        MH = M4 // 2
        for t in range(S):
            s, sl = divmod(t, SWEEP)
            if s + 1 < n_sweeps and sl == 0:
                load_x(s + 1)
                new_ring(s + 1)

            rb = ring_bufs[s]
            # Three PSUM tiles (one bank each): A holds gates g,i (m 0-7),
            # B holds f (m 8-11), C holds o (m 12-15).  Separate banks mean
            # each tile's activations wait only on that tile's matmuls.
            gpa = psga.tile([P, MH, B], f32, tag="ga")
            gpb = psgb.tile([P, KH, B], f32, tag="gb")
            gpc = psgc.tile([P, KH, B], f32, tag="gc")

            def gp_slot(m):
                if m < MH:
                    return gpa, m, MH
                if m < MH + KH:
                    return gpb, m - MH, KH
                return gpc, m - MH - KH, KH

            # PSUM accumulation groups are 2KB-bank granular: start=True marks
            # the bank lazily-zero (first writer of each byte overwrites, later
            # writers accumulate); stop goes on the bank's last matmul.
            first_pre = None
            for m in range(M4):
                gp_t, ml, nl = gp_slot(m)
                mm = nc.tensor.matmul(gp_t[:, ml, :], id_sb[:], rb[:, sl, m, :],
                                      start=(ml == 0),
                                      stop=(t == 0 and ml == nl - 1))
                if m == 0:
                    first_pre = mm
            if last_sweep_mm is not None:
                # keep the PE stream interleaved: this step's preloads run
                # after the previous step's sweep work (ordering-only)
                add_dep_helper(first_pre.ins, last_sweep_mm.ins, sync=False,
                               reason="preloads after prior step sweeps")
            last_h_mm = first_pre
            if t > 0:
                for m in range(M4):
                    gp_t, ml, nl = gp_slot(m)
                    for k in range(KH):
                        last_h_mm = nc.tensor.matmul(
                            gp_t[:, ml, :], wh_sb[:, k, m, :], h_prev[:, k, :],
                            start=False,
                            stop=(k == KH - 1 and ml == nl - 1))

            # elementwise cell update; gate m-chunk order is [g, i | f | o]
            tg = ewp.tile([P, KH, B], bf16, tag="tg")
            nc.scalar.activation(tg[:], gpa[:, 0:KH, :], AF.Tanh)
            si = ewp.tile([P, KH, B], bf16, tag="si")
            nc.scalar.activation(si[:], gpa[:, KH:2 * KH, :], AF.Sigmoid)
            t1 = ewp.tile([P, KH, B], f32, tag="t1")
            nc.vector.tensor_mul(out=t1[:], in0=si[:], in1=tg[:])

            sf = ewp.tile([P, KH, B], bf16, tag="sf")
            nc.scalar.activation(sf[:], gpb[:], AF.Sigmoid)
            so = ewp.tile([P, KH, B], bf16, tag="so")
            nc.scalar.activation(so[:], gpc[:], AF.Sigmoid)

            c_new = statep.tile([P, KH, B], f32, tag="c")
            if t == 0:
                nc.vector.tensor_copy(out=c_new[:], in_=t1[:])
            else:
                t2 = ewp.tile([P, KH, B], f32, tag="t2")
                nc.vector.tensor_mul(out=t2[:], in0=sf[:], in1=c_prev[:])
                nc.vector.tensor_add(out=c_new[:], in0=t1[:], in1=t2[:])
            tct = ewp.tile([P, KH, B], bf16, tag="tct")
            tct_inst = nc.scalar.activation(tct[:], c_new[:], AF.Tanh)
            h_new = statep.tile([P, KH, B], bf16, tag="hT")
            hmul_inst = nc.vector.tensor_mul(out=h_new[:], in0=so[:], in1=tct[:])
            nc.sync.dma_start(out=hsT.ap()[t].rearrange("k p b -> p k b"), in_=h_new[:])

            h_prev, c_prev = h_new, c_new

            # emit next window's x-projection groups, pinned behind this step's
            # last recurrent matmul so they fill this step's PE idle tail
            if s + 1 < n_sweeps:
                for j, g in enumerate(range(sl * gps, (sl + 1) * gps)):
                    dve = (j % 2 == 0)
                    last_sweep_mm = sweep_group(
                        s + 1, g // M4, g % M4, after=last_h_mm,
                        evict_dve=dve,
                        evict_after=(hmul_inst if dve else tct_inst))
            else:
                last_sweep_mm = None

    nc.compile()
    return nc


def _get_nc(S, SWEEP):
    key = (S, SWEEP)
    if key not in _NC_CACHE:
        _NC_CACHE[key] = build(S, SWEEP)
    return _NC_CACHE[key]


def prep_core_inputs(x, Wc, bc, Wi, bi, Wf, bf, Wo, bo, reverse):
    """Pack one direction's inputs into the kernel's layouts. x: (B, S, I) f32."""
    bft = ml_dtypes.bfloat16
    if reverse:
        x = x[:, ::-1, :]
    S = x.shape[1]
    Wcat = np.concatenate([Wc, Wi, Wf, Wo], axis=1)      # (I+H, 4H), gate order [g,i,f,o]
    bcat = np.concatenate([bc, bi, bf, bo]).astype(np.float32)
    Wx, Wh = Wcat[:ID], Wcat[ID:]

    xT = (
        x.transpose(2, 1, 0)                  # (I, S, B)
        .reshape(KI, P, S * B)
        .transpose(1, 0, 2)                   # (P, KI, S*B)
    )
    wxp = Wx.reshape(KI, P, M4, P).transpose(1, 0, 2, 3)
    whp = Wh.reshape(KH, P, M4, P).transpose(1, 0, 2, 3)
    biasp = bcat.reshape(M4, P).T
    return {
        "xT": np.ascontiguousarray(xT).astype(bft),
        "wx": np.ascontiguousarray(wxp).astype(bft),
        "wh": np.ascontiguousarray(whp).astype(bft),
        "bias": np.ascontiguousarray(biasp),
        "ident": np.eye(P, dtype=bft),
    }


def run_lstm(x, Wi_f, bi_f, Wf_f, bf_f, Wc_f, bc_f, Wo_f, bo_f,
             Wi_b, bi_b, Wf_b, bf_b, Wc_b, bc_b, Wo_b, bo_b,
             trace=False, trace_cores=None):
    from concourse import bass_utils

    x = np.asarray(x, dtype=np.float32)
    S = x.shape[1]
    nc = _get_nc(S, SWEEP_FULL if S % SWEEP_FULL == 0 else S)
    im0 = prep_core_inputs(x, Wc_f, bc_f, Wi_f, bi_f, Wf_f, bf_f, Wo_f, bo_f, False)
    im1 = prep_core_inputs(x, Wc_b, bc_b, Wi_b, bi_b, Wf_b, bf_b, Wo_b, bo_b, True)
    res = bass_utils.run_bass_kernel_spmd(
        nc, [im0, im1], core_ids=[0, 1], trace=trace, trace_cores=trace_cores,
    )
    hsf = res.results[0]["hsT"].astype(np.float32)    # (S, KH, P, B)
    hsb = res.results[1]["hsT"][::-1].astype(np.float32)
    fwd = hsf.transpose(0, 3, 1, 2).reshape(S, B, HD)   # (S, B, H)
    bwd = hsb.transpose(0, 3, 1, 2).reshape(S, B, HD)
    out = np.concatenate([fwd, bwd], axis=2).transpose(1, 0, 2)  # (B, S, 2H)
    return np.ascontiguousarray(out), res


def kernel(x, Wi_f, bi_f, Wf_f, bf_f, Wc_f, bc_f, Wo_f, bo_f,
           Wi_b, bi_b, Wf_b, bf_b, Wc_b, bc_b, Wo_b, bo_b):
    out, _ = run_lstm(x, Wi_f, bi_f, Wf_f, bf_f, Wc_f, bc_f, Wo_f, bo_f,
                      Wi_b, bi_b, Wf_b, bf_b, Wc_b, bc_b, Wo_b, bo_b)
    return out


# revision 13
# speedup vs baseline: 1.1037x; 1.0015x over previous
"""Bidirectional LSTM kernel for Trainium2 (Bass/Tile), B=64 S=256 I=H=512.

Strategy:
- Core 0 runs the forward direction, core 1 the backward direction (same SPMD
  program; the host reverses time for core 1's inputs/outputs).
- Transposed ("gates^T") layout: the recurrent GEMM keeps the 64 Wh weight
  tiles stationary on the PE array and streams h^T (512x64) as the moving
  operand, producing gates^T (2048x64) in PSUM.  The elementwise cell update
  then runs on full 128-partition tiles and produces h^T directly in the
  layout the next step's GEMM consumes - no per-step transpose.
- The input projection x@Wx + b is computed in 16-step sweep windows into an
  SBUF ring buffer (amortized weight loads), and injected into each step's
  PSUM accumulation via identity-matmul preloads, so the per-step critical
  path never touches it.
"""

import numpy as np
import ml_dtypes

P = 128
B = 64          # batch
HD = 512        # hidden dim
ID = 512        # input dim
KH = HD // P    # 4 k-chunks over h
KI = ID // P    # 4 k-chunks over x
M4 = 4 * HD // P  # 16 m-chunks over the 4*H gate dim; order [i, g, f, o]
S_FULL = 256
SWEEP_FULL = 16

_NC_CACHE = {}


def build(S=S_FULL, SWEEP=SWEEP_FULL):
    """Build and bacc-compile the single-core LSTM program."""
    import concourse.bacc as bacc
    import concourse.mybir as mybir
    import concourse.tile as tile
    from concourse.tile import add_dep_helper
    from contextlib import ExitStack

    AF = mybir.ActivationFunctionType
    bf16 = mybir.dt.bfloat16
    f32 = mybir.dt.float32

    assert S % SWEEP == 0
    n_sweeps = S // SWEEP
    COLS = SWEEP * B              # columns per sweep window
    NCH = max(1, COLS // 512)     # 512-col chunks per window
    NCOL = COLS // NCH            # columns per chunk (<= 512)
    TPC = NCOL // B               # timesteps covered per chunk
    n_groups = NCH * M4           # (n, m) GEMM groups per window
    assert n_groups % SWEEP == 0
    gps = n_groups // SWEEP       # groups emitted per step

    nc = bacc.Bacc("TRN2", target_bir_lowering=False, debug=False, num_devices=2)

    xT = nc.dram_tensor("xT", (P, KI, S * B), bf16, kind="ExternalInput")
    wx = nc.dram_tensor("wx", (P, KI, M4, P), bf16, kind="ExternalInput")
    wh = nc.dram_tensor("wh", (P, KH, M4, P), bf16, kind="ExternalInput")
    bias = nc.dram_tensor("bias", (P, M4), f32, kind="ExternalInput")
    ident = nc.dram_tensor("ident", (P, P), bf16, kind="ExternalInput")
    hsT = nc.dram_tensor("hsT", (S, KH, P, B), bf16, kind="ExternalOutput")

    with tile.TileContext(nc) as tc, ExitStack() as ctx:
        constp = ctx.enter_context(tc.tile_pool(name="const", bufs=1))
        xinp = ctx.enter_context(tc.tile_pool(name="xin", bufs=2))
        ringp = ctx.enter_context(tc.tile_pool(name="ring", bufs=2))
        statep = ctx.enter_context(tc.tile_pool(name="state", bufs=3))
        ewp = ctx.enter_context(tc.tile_pool(name="ew", bufs=3))
        psga = ctx.enter_context(tc.tile_pool(name="psum_ga", bufs=2, space="PSUM"))
        psgb = ctx.enter_context(tc.tile_pool(name="psum_gb", bufs=1, space="PSUM"))
        psgc = ctx.enter_context(tc.tile_pool(name="psum_gc", bufs=1, space="PSUM"))
        psx = ctx.enter_context(tc.tile_pool(name="psum_x", bufs=4, space="PSUM"))

        wx_sb = constp.tile([P, KI, M4, P], bf16)
        wh_sb = constp.tile([P, KH, M4, P], bf16)
        for k in range(KI):
            nc.sync.dma_start(out=wx_sb[:, k], in_=wx.ap()[:, k])
        for k in range(KH):
            nc.sync.dma_start(out=wh_sb[:, k], in_=wh.ap()[:, k])
        bias_sb = constp.tile([P, M4], f32)
        nc.sync.dma_start(out=bias_sb[:], in_=bias.ap())
        id_sb = constp.tile([P, P], bf16)
        nc.sync.dma_start(out=id_sb[:], in_=ident.ap())

        x_bufs = {}
        ring_bufs = {}

        def load_x(s):
            t_ = xinp.tile([P, KI, COLS], bf16, tag="xin", name=f"xin{s}")
            nc.sync.dma_start(out=t_[:], in_=xT.ap()[:, :, s * COLS:(s + 1) * COLS])
            x_bufs[s] = t_

        def new_ring(s):
            ring_bufs[s] = ringp.tile([P, SWEEP, M4, B], bf16, tag="ring", name=f"ring{s}")

        def sweep_group(s, n, m, after=None, evict_dve=False, evict_after=None):
            # x-projection GEMM for sweep window s, column-chunk n, m-chunk m.
            # `after`: PE instruction to order the first matmul behind
            # (ordering-only dep, same engine) so sweeps land in step tails.
            xb = x_bufs[s]
            rb = ring_bufs[s]
            pt = psx.tile([P, TPC, B], f32, tag="psx")
            last = None
            for k in range(KI):
                mm = nc.tensor.matmul(
                    pt[:], wx_sb[:, k, m, :], xb[:, k, n * NCOL:(n + 1) * NCOL],
                    start=(k == 0), stop=(k == KI - 1),
                )
                if k == 0 and after is not None:
                    add_dep_helper(mm.ins, after.ins, sync=False,
                                   reason="pin sweep into step tail")
                last = mm
            # evict to ring with the gate bias folded in (per-partition bias);
            # alternate between DVE and ScalarE to balance engine load
            if evict_dve:
                ev = nc.vector.tensor_scalar_add(
                    out=rb[:, n * TPC:(n + 1) * TPC, m, :], in0=pt[:],
                    scalar1=bias_sb[:, m:m + 1],
                )
            else:
                ev = nc.scalar.activation(
                    rb[:, n * TPC:(n + 1) * TPC, m, :], pt[:],
                    AF.Identity, bias=bias_sb[:, m:m + 1],
                )
            if evict_after is not None:
                add_dep_helper(ev.ins, evict_after.ins, sync=False,
                               reason="evict after step chain ops")
            return last

        # prologue: first sweep window fully
        load_x(0)
        new_ring(0)
        for n in range(NCH):
            for m in range(M4):
                sweep_group(0, n, m)

        h_prev = None
        c_prev = None
        last_sweep_mm = None   # last sweep matmul of the previous step
        MH = M4 // 2
        for t in range(S):
            s, sl = divmod(t, SWEEP)
            if s + 1 < n_sweeps and sl == 0:
                load_x(s + 1)
                new_ring(s + 1)

            rb = ring_bufs[s]
            # Three PSUM tiles (one bank each): A holds gates g,i (m 0-7),
            # B holds f (m 8-11), C holds o (m 12-15).  Separate banks mean
            # each tile's activations wait only on that tile's matmuls.
            gpa = psga.tile([P, MH, B], f32, tag="ga")
            gpb = psgb.tile([P, KH, B], f32, tag="gb")
            gpc = psgc.tile([P, KH, B], f32, tag="gc")

            def gp_slot(m):
                if m < MH:
                    return gpa, m, MH
                if m < MH + KH:
                    return gpb, m - MH, KH
                return gpc, m - MH - KH, KH

            # PSUM accumulation groups are 2KB-bank granular: start=True marks
            # the bank lazily-zero (first writer of each byte overwrites, later
            # writers accumulate); stop goes on the bank's last matmul.
            first_pre = nc.tensor.matmul(
                gpa[:], id_sb[:], rb[:, sl, 0:MH, :],
                start=True, stop=(t == 0))
            nc.tensor.matmul(gpb[:], id_sb[:], rb[:, sl, MH:MH + KH, :],
                             start=True, stop=(t == 0))
            nc.tensor.matmul(gpc[:], id_sb[:], rb[:, sl, MH + KH:M4, :],
                             start=True, stop=(t == 0))
            if last_sweep_mm is not None:
                # keep the PE stream interleaved: this step's preloads run
                # after the previous step's sweep work (ordering-only)
                add_dep_helper(first_pre.ins, last_sweep_mm.ins, sync=False,
                               reason="preloads after prior step sweeps")
            last_h_mm = first_pre
            if t > 0:
                for m in range(M4):
                    gp_t, ml, nl = gp_slot(m)
                    for k in range(KH):
                        last_h_mm = nc.tensor.matmul(
                            gp_t[:, ml, :], wh_sb[:, k, m, :], h_prev[:, k, :],
                            start=False,
                            stop=(k == KH - 1 and ml == nl - 1))

            # elementwise cell update; gate m-chunk order is [i, g | f | o]
            si = ewp.tile([P, KH, B], bf16, tag="si")
            nc.scalar.activation(si[:], gpa[:, 0:KH, :], AF.Sigmoid)
            tg = ewp.tile([P, KH, B], bf16, tag="tg")
            nc.scalar.activation(tg[:], gpa[:, KH:2 * KH, :], AF.Tanh)
            t1 = ewp.tile([P, KH, B], f32, tag="t1")
            nc.vector.tensor_mul(out=t1[:], in0=si[:], in1=tg[:])

            sf = ewp.tile([P, KH, B], bf16, tag="sf")
            nc.scalar.activation(sf[:], gpb[:], AF.Sigmoid)
            so = ewp.tile([P, KH, B], bf16, tag="so")
            nc.scalar.activation(so[:], gpc[:], AF.Sigmoid)

            c_new = statep.tile([P, KH, B], f32, tag="c")
            if t == 0:
                nc.vector.tensor_copy(out=c_new[:], in_=t1[:])
            else:
                t2 = ewp.tile([P, KH, B], f32, tag="t2")
                nc.vector.tensor_mul(out=t2[:], in0=sf[:], in1=c_prev[:])
                nc.vector.tensor_add(out=c_new[:], in0=t1[:], in1=t2[:])
            tct = ewp.tile([P, KH, B], bf16, tag="tct")
            tct_inst = nc.scalar.activation(tct[:], c_new[:], AF.Tanh)
            h_new = statep.tile([P, KH, B], bf16, tag="hT")
            hmul_inst = nc.vector.tensor_mul(out=h_new[:], in0=so[:], in1=tct[:])
            nc.sync.dma_start(out=hsT.ap()[t].rearrange("k p b -> p k b"), in_=h_new[:])

            h_prev, c_prev = h_new, c_new

            # emit next window's x-projection groups, pinned behind this step's
            # last recurrent matmul so they fill this step's PE idle tail
            if s + 1 < n_sweeps:
                for j, g in enumerate(range(sl * gps, (sl + 1) * gps)):
                    dve = (j % 2 == 0)
                    last_sweep_mm = sweep_group(
                        s + 1, g // M4, g % M4, after=last_h_mm,
                        evict_dve=dve,
                        evict_after=(hmul_inst if dve else tct_inst))
            else:
                last_sweep_mm = None

    nc.compile()
    return nc


def _get_nc(S, SWEEP):
    key = (S, SWEEP)
    if key not in _NC_CACHE:
        _NC_CACHE[key] = build(S, SWEEP)
    return _NC_CACHE[key]


def prep_core_inputs(x, Wc, bc, Wi, bi, Wf, bf, Wo, bo, reverse):
    """Pack one direction's inputs into the kernel's layouts. x: (B, S, I) f32."""
    bft = ml_dtypes.bfloat16
    if reverse:
        x = x[:, ::-1, :]
    S = x.shape[1]
    Wcat = np.concatenate([Wi, Wc, Wf, Wo], axis=1)      # (I+H, 4H), gate order [i,g,f,o]
    bcat = np.concatenate([bi, bc, bf, bo]).astype(np.float32)
    Wx, Wh = Wcat[:ID], Wcat[ID:]

    xT = (
        x.transpose(2, 1, 0)                  # (I, S, B)
        .reshape(KI, P, S * B)
        .transpose(1, 0, 2)                   # (P, KI, S*B)
    )
    wxp = Wx.reshape(KI, P, M4, P).transpose(1, 0, 2, 3)
    whp = Wh.reshape(KH, P, M4, P).transpose(1, 0, 2, 3)
    biasp = bcat.reshape(M4, P).T
    return {
        "xT": np.ascontiguousarray(xT).astype(bft),
        "wx": np.ascontiguousarray(wxp).astype(bft),
        "wh": np.ascontiguousarray(whp).astype(bft),
        "bias": np.ascontiguousarray(biasp),
        "ident": np.eye(P, dtype=bft),
    }


def run_lstm(x, Wi_f, bi_f, Wf_f, bf_f, Wc_f, bc_f, Wo_f, bo_f,
             Wi_b, bi_b, Wf_b, bf_b, Wc_b, bc_b, Wo_b, bo_b,
             trace=False, trace_cores=None):
    from concourse import bass_utils

    x = np.asarray(x, dtype=np.float32)
    S = x.shape[1]
    nc = _get_nc(S, SWEEP_FULL if S % SWEEP_FULL == 0 else S)
    im0 = prep_core_inputs(x, Wc_f, bc_f, Wi_f, bi_f, Wf_f, bf_f, Wo_f, bo_f, False)
    im1 = prep_core_inputs(x, Wc_b, bc_b, Wi_b, bi_b, Wf_b, bf_b, Wo_b, bo_b, True)
    res = bass_utils.run_bass_kernel_spmd(
        nc, [im0, im1], core_ids=[0, 1], trace=trace, trace_cores=trace_cores,
    )
    hsf = res.results[0]["hsT"].astype(np.float32)    # (S, KH, P, B)
    hsb = res.results[1]["hsT"][::-1].astype(np.float32)
    fwd = hsf.transpose(0, 3, 1, 2).reshape(S, B, HD)   # (S, B, H)
    bwd = hsb.transpose(0, 3, 1, 2).reshape(S, B, HD)
    out = np.concatenate([fwd, bwd], axis=2).transpose(1, 0, 2)  # (B, S, 2H)
    return np.ascontiguousarray(out), res


def kernel(x, Wi_f, bi_f, Wf_f, bf_f, Wc_f, bc_f, Wo_f, bo_f,
           Wi_b, bi_b, Wf_b, bf_b, Wc_b, bc_b, Wo_b, bo_b):
    out, _ = run_lstm(x, Wi_f, bi_f, Wf_f, bf_f, Wc_f, bc_f, Wo_f, bo_f,
                      Wi_b, bi_b, Wf_b, bf_b, Wc_b, bc_b, Wo_b, bo_b)
    return out
